# revision 1
# baseline (speedup 1.0000x reference)
"""Trainium2 Bass kernel for DirCFConv-style GNN message passing.

Computes, for inputs s:(B,N,H) f32, ef_mask:(B,N,N,H) f32, W:(H,H), b:(H,):
    m   = SiLU(LayerNorm(s @ W.T + b))          # (B,N,H)
    out[b,i,h] = sum_j ef_mask[b,i,j,h] * m[b,j,h]

Sharding: 8 cores, core c handles batch b = c // 2 and query-node half
i in [ (c%2)*256, (c%2)*256+256 ).  Each core streams its 64 MiB mask
shard from HBM (memory-bound), multiplies tiles [j=128p, 16i, 128h] by a
replicated m tile on the vector engine, and reduces over the j partition
axis with a ones-vector matmul accumulating in PSUM.
"""

import numpy as np

import concourse.bass as bass
import concourse.bacc as bacc
import concourse.tile as tile
from concourse import mybir
from concourse.bass_utils import run_bass_kernel_spmd
from concourse.masks import make_identity

B, N, H = 4, 512, 128
P = 128
NJT = N // P          # 4 j-tiles of 128
ISUB = 16             # i's per mask tile -> 1 MiB DMAs
IH = N // 2           # 256 i's per core
N_CORES = 8
LN_EPS = 1e-5
F32 = mybir.dt.float32


def build_nc(ih=IH, prod_dtype=F32, repeat=1, do_mul=True, do_mm=True):
    nc = bacc.Bacc()
    s_d = nc.declare_dram_parameter("s", [N, H], F32, isOutput=False)
    w_d = nc.declare_dram_parameter("w", [H, H], F32, isOutput=False)
    b_d = nc.declare_dram_parameter("b", [H], F32, isOutput=False)
    mask_d = nc.declare_dram_parameter("mask", [ih, N, H], F32, isOutput=False)
    out_d = nc.declare_dram_parameter("out", [ih, H], F32, isOutput=True)

    nit = ih // ISUB
    with tile.TileContext(nc) as tc:
        with (
            tc.tile_pool(name="consts", bufs=1) as consts,
            tc.tile_pool(name="small", bufs=4) as small,
            tc.tile_pool(name="loads", bufs=10) as loads,
            tc.tile_pool(name="outs", bufs=3) as outs,
        ):
            stage1_psum = tc.tile_pool(name="spsum", bufs=1, space="PSUM")
            spsum = stage1_psum.__enter__()
            # ---------------- constants ----------------
            # All constants are produced on gpsimd BEFORE make_identity so the
            # single carrier wait (Pool sem) covers every one of them.
            ones_col = consts.tile([P, 1], F32)
            nc.gpsimd.memset(ones_col, 1.0)
            ones_row = consts.tile([1, P], F32)
            nc.gpsimd.memset(ones_row, 1.0)
            # eps on DVE: its consumer (ACT Sqrt) already waits on DVE for mv,
            # and one DVE sem wait covers both (Activation also allows only 1).
            eps_t = consts.tile([P, 1], F32)
            nc.vector.memset(eps_t, LN_EPS)
            ident = consts.tile([P, P], F32)
            make_identity(nc, ident)

            w_sb = consts.tile([H, H], F32)
            nc.sync.dma_start(out=w_sb, in_=w_d[:, :])
            bias_sb = consts.tile([1, H], F32)
            b_ap = b_d[:]
            bias_src = bass.AP(
                tensor=b_ap.tensor, offset=b_ap.offset, ap=[[0, 1]] + list(b_ap.ap)
            )
            nc.sync.dma_start(out=bias_sb, in_=bias_src)

            # Wait-carrier: walrus allows only ONE sync wait per Matmult, so
            # absorb the gpsimd(identity) dependency into a throwaway PE op;
            # later matmuls then only carry their own single DMA/engine wait.
            carrier_ps = spsum.tile([P, P], F32)
            nc.tensor.transpose(carrier_ps, ident, ident)

            # W^T via PE-transpose: (o,h) -> (h,o)
            wT_ps = spsum.tile([H, H], F32)
            nc.tensor.transpose(wT_ps, w_sb, ident)
            wT_sb = consts.tile([H, H], F32)
            nc.scalar.copy(wT_sb, wT_ps)

            # ------------- m = SiLU(LN(s @ W.T + b)) -------------
            # All four s^T blocks share one PSUM bank (one zero-region group);
            # likewise the four h = s@W.T+b blocks.  No PSUM slot rotation ->
            # no extra release waits on any Matmult.
            sT_all = spsum.tile([P, NJT * P], F32)
            h_all = spsum.tile([P, NJT * H], F32)
            s_sbs = []
            for jt in range(NJT):
                s_sb = small.tile([P, H], F32, tag=f"s_sb{jt}")
                nc.sync.dma_start(out=s_sb, in_=s_d[jt * P:(jt + 1) * P, :])
                s_sbs.append(s_sb)
                nc.tensor.matmul(
                    sT_all[:, jt * P:(jt + 1) * P],
                    lhsT=s_sb,
                    rhs=ident,
                    is_transpose=True,
                    start=(jt == 0),
                    stop=(jt == NJT - 1),
                )
            sT_sb = consts.tile([P, NJT * P], F32)
            nc.scalar.copy(sT_sb, sT_all)
            for jt in range(NJT):
                nc.tensor.matmul(
                    h_all[:, jt * H:(jt + 1) * H],
                    lhsT=sT_sb[:, jt * P:(jt + 1) * P],
                    rhs=wT_sb,
                    start=(jt == 0),
                    stop=False,
                )
                nc.tensor.matmul(
                    h_all[:, jt * H:(jt + 1) * H],
                    lhsT=ones_row,
                    rhs=bias_sb,
                    start=False,
                    stop=(jt == NJT - 1),
                )

            # m_rep[:, jt, r, :] = m[jt*128:(jt+1)*128, :] for every r (ISUB copies)
            m_rep = consts.tile([P, NJT, ISUB, H], F32)
            for jt in range(NJT):
                h_ps = h_all[:, jt * H:(jt + 1) * H]
                stats = small.tile([P, 6], F32)
                nc.vector.bn_stats(stats, h_ps)
                mv = small.tile([P, 2], F32)
                nc.vector.bn_aggr(mv, stats)
                xc = small.tile([P, H], F32)
                nc.vector.tensor_scalar_sub(xc, h_ps, mv[:, 0:1])
                stdv = small.tile([P, 1], F32)
                nc.scalar.activation(
                    stdv, mv[:, 1:2], mybir.ActivationFunctionType.Sqrt, bias=eps_t
                )
                rstd = small.tile([P, 1], F32)
                nc.vector.reciprocal(rstd, stdv)
                xn = small.tile([P, H], F32)
                nc.vector.tensor_scalar_mul(xn, xc, rstd)
                sg = small.tile([P, H], F32)
                nc.scalar.activation(sg, xn, mybir.ActivationFunctionType.Sigmoid)
                nc.vector.tensor_mul(m_rep[:, jt, 0, :], xn, sg)
                rep = 1
                while rep < ISUB:
                    cnt = min(rep, ISUB - rep)
                    nc.vector.tensor_copy(
                        m_rep[:, jt, rep:rep + cnt, :], m_rep[:, jt, 0:cnt, :]
                    )
                    rep += cnt

            # stage-1 PSUM pools stay open: releasing them would put a
            # (PE+DVE) release-wait on stage-2's first Matmult, which walrus
            # cannot encode.  4 stage-1 banks + 1 acc + 2 transpose banks = 7.
            # ------------- out[i,h] = sum_j mask[i,j,h] * m[j,h] -------------
            # acc2[h, i] += pr[:, i, :].T @ ones  (partition-reduce over j via PE;
            # product is the stationary operand so moving rows n=1 and plain f32
            # carries no 4-cyc/row penalty). One PSUM bank holds all ih columns.
            opsum_cm = tc.tile_pool(name="opsum", bufs=1, space="PSUM")
            opsum = opsum_cm.__enter__()
            tpsum_cm = tc.tile_pool(name="tpsum", bufs=2, space="PSUM")
            tpsum = tpsum_cm.__enter__()
            acc2 = opsum.tile([P, ih], F32)
            for rp in range(repeat):
              for it in range(nit):
                for jt in range(NJT):
                    mt = loads.tile([P, ISUB, H], F32)
                    src = mask_d[
                        it * ISUB:(it + 1) * ISUB, jt * P:(jt + 1) * P, :
                    ].rearrange("i j h -> j i h")
                    nc.sync.dma_start(out=mt, in_=src)
                    # in-place multiply: product overwrites the mask tile
                    if do_mul:
                        nc.vector.tensor_mul(mt, mt, m_rep[:, jt])
                    for ii in range(ISUB if do_mm else 0):
                        i = it * ISUB + ii
                        # One accumulation group spans the whole bank: start
                        # zeroes the full 2KB zero region, so only the global
                        # first/last matmuls carry start/stop.
                        nc.tensor.matmul(
                            acc2[:, i:i + 1],
                            lhsT=mt[:, ii, :],
                            rhs=ones_col,
                            start=(it == 0 and jt == 0 and ii == 0),
                            stop=(
                                it == nit - 1 and jt == NJT - 1 and ii == ISUB - 1
                            ),

                        )
            # epilogue: acc2 is [h, i]; transpose 128-blocks back to [i, h]
            accT = outs.tile([P, ih], F32)
            if do_mm:
                nc.vector.tensor_copy(accT, acc2)
            else:
                nc.vector.memset(accT, 0.0)
                nc.vector.tensor_copy(acc2[:, 0:1], accT[:, 0:1])
            for blk in range(ih // P):
                tp = tpsum.tile([P, P], F32)
                nc.tensor.transpose(tp, accT[:, blk * P:(blk + 1) * P], ident)
                oT = outs.tile([P, P], F32)
                nc.scalar.copy(oT, tp)
                nc.sync.dma_start(out=out_d[blk * P:(blk + 1) * P, :], in_=oT)
            tpsum_cm.__exit__(None, None, None)
            opsum_cm.__exit__(None, None, None)
            stage1_psum.__exit__(None, None, None)
    nc.finalize()
    return nc


_NC_CACHE = {}


def _get_nc():
    key = "main"
    if key not in _NC_CACHE:
        _NC_CACHE[key] = build_nc()
    return _NC_CACHE[key]


def kernel(s, ef_mask, W, b):
    s = np.ascontiguousarray(s, dtype=np.float32)
    ef_mask = np.ascontiguousarray(ef_mask, dtype=np.float32)
    W = np.ascontiguousarray(W, dtype=np.float32)
    b = np.ascontiguousarray(b, dtype=np.float32)

    nc = _get_nc()
    in_maps = []
    for c in range(N_CORES):
        bb = c // 2
        half = c % 2
        in_maps.append(
            {
                "s": s[bb],
                "w": W,
                "b": b,
                "mask": ef_mask[bb, half * IH:(half + 1) * IH],
            }
        )
    res = run_bass_kernel_spmd(nc, in_maps, list(range(N_CORES))).results
    out = np.empty((B, N, H), dtype=np.float32)
    for c in range(N_CORES):
        bb = c // 2
        half = c % 2
        out[bb, half * IH:(half + 1) * IH] = res[c]["out"]
    return out



# revision 6
# speedup vs baseline: 1.7598x; 1.7598x over previous
"""Trainium2 Bass kernel for DirCFConv-style GNN message passing.

Computes, for inputs s:(B,N,H) f32, ef_mask:(B,N,N,H) f32, W:(H,H), b:(H,):
    m   = SiLU(LayerNorm(s @ W.T + b))          # (B,N,H)
    out[b,i,h] = sum_j ef_mask[b,i,j,h] * m[b,j,h]

Sharding: 8 cores, core c handles batch b = c // 2 and query-node half
i in [ (c%2)*256, (c%2)*256+256 ).  The 64 MiB mask shard is laid out
[j, i, h] (transposed during host-side sharding) so every mask DMA is
128 partitions x 32 KiB fully-contiguous lines (~full HBM bandwidth).
Per i-chunk of 16, one 4 MiB DMA brings in all j; the vector engine
multiplies by a replicated m tile; the tensor engine column-sum-reduces
over the j partition axis with a ones-selector matmul whose output
partition k holds chunk k, accumulating all chunks in one PSUM group.
"""

import numpy as np

import concourse.bass as bass
import concourse.bacc as bacc
import concourse.tile as tile
from concourse import mybir
from concourse.bass_utils import run_bass_kernel_spmd
from concourse.masks import make_identity

B, N, H = 4, 512, 128
P = 128
NJT = N // P          # 4 j-tiles of 128 partitions
IC = 16               # i's per chunk -> 4 MiB DMAs, PSUM partition k = chunk k
IH = N // 2           # 256 i's per core
N_CORES = 8
LN_EPS = 1e-5
F32 = mybir.dt.float32
CH = IC * H           # 2048 columns per (jt, chunk)
MMF = 512             # moving-operand columns per matmul (fp32 max)


def build_nc(ih=IH):
    nc = bacc.Bacc()
    s_d = nc.declare_dram_parameter("s", [N, H], F32, isOutput=False)
    w_d = nc.declare_dram_parameter("w", [H, H], F32, isOutput=False)
    b_d = nc.declare_dram_parameter("b", [H], F32, isOutput=False)
    # mask is the [j, i, h]-transposed shard
    mask_d = nc.declare_dram_parameter("mask", [N, ih, H], F32, isOutput=False)
    out_d = nc.declare_dram_parameter("out", [ih, H], F32, isOutput=True)

    nch = ih // IC        # i-chunks; chunk k lands on PSUM partition k
    with tile.TileContext(nc) as tc:
        with (
            tc.tile_pool(name="consts", bufs=1) as consts,
            tc.tile_pool(name="small", bufs=4) as small,
            tc.tile_pool(name="loads", bufs=3) as loads,
            tc.tile_pool(name="outs", bufs=1) as outs,
        ):
            stage1_psum = tc.tile_pool(name="spsum", bufs=1, space="PSUM")
            spsum = stage1_psum.__enter__()
            # ---------------- constants ----------------
            # All constants are produced on gpsimd BEFORE make_identity so the
            # single carrier wait (Pool sem) covers every one of them.
            ones_row = consts.tile([1, P], F32)
            nc.gpsimd.memset(ones_row, 1.0)
            # sel[:, k*nch:(k+1)*nch] is a one-hot stationary operand routing
            # chunk k's column-sum to PSUM partition k (zeros to the others,
            # keeping every matmul's footprint the full [nch, MMF] region).
            sel = consts.tile([P, nch * nch], F32)
            nc.gpsimd.memset(sel, 0.0)
            for k in range(nch):
                nc.gpsimd.memset(sel[:, k * nch + k:k * nch + k + 1], 1.0)
            # eps on DVE: its consumer (ACT Sqrt) already waits on DVE for mv,
            # and one DVE sem wait covers both (Activation also allows only 1).
            eps_t = consts.tile([P, 1], F32)
            nc.vector.memset(eps_t, LN_EPS)
            ident = consts.tile([P, P], F32)
            make_identity(nc, ident)

            w_sb = consts.tile([H, H], F32)
            nc.sync.dma_start(out=w_sb, in_=w_d[:, :])
            bias_sb = consts.tile([1, H], F32)
            b_ap = b_d[:]
            bias_src = bass.AP(
                tensor=b_ap.tensor, offset=b_ap.offset, ap=[[0, 1]] + list(b_ap.ap)
            )
            nc.sync.dma_start(out=bias_sb, in_=bias_src)

            # Wait-carrier: walrus allows only ONE sync wait per Matmult, so
            # absorb the gpsimd(memsets) dependency into a throwaway PE op;
            # later matmuls then only carry their own single DMA/engine wait.
            carrier_ps = spsum.tile([P, P], F32)
            nc.tensor.transpose(carrier_ps, ident, ident)

            # W^T via PE-transpose: (o,h) -> (h,o)
            wT_ps = spsum.tile([H, H], F32)
            nc.tensor.transpose(wT_ps, w_sb, ident)
            wT_sb = consts.tile([H, H], F32)
            nc.scalar.copy(wT_sb, wT_ps)

            # ------------- m = SiLU(LN(s @ W.T + b)) -------------
            # All four s^T blocks share one PSUM bank (one zero-region group);
            # likewise the four h = s@W.T+b blocks.  No PSUM slot rotation ->
            # no extra release waits on any Matmult.
            sT_all = spsum.tile([P, NJT * P], F32)
            h_all = spsum.tile([P, NJT * H], F32)
            s_sbs = []
            for jt in range(NJT):
                s_sb = small.tile([P, H], F32, tag=f"s_sb{jt}")
                nc.sync.dma_start(out=s_sb, in_=s_d[jt * P:(jt + 1) * P, :])
                s_sbs.append(s_sb)
                nc.tensor.matmul(
                    sT_all[:, jt * P:(jt + 1) * P],
                    lhsT=s_sb,
                    rhs=ident,
                    is_transpose=True,
                    start=(jt == 0),
                    stop=(jt == NJT - 1),
                )
            sT_sb = consts.tile([P, NJT * P], F32)
            nc.scalar.copy(sT_sb, sT_all)
            for jt in range(NJT):
                nc.tensor.matmul(
                    h_all[:, jt * H:(jt + 1) * H],
                    lhsT=sT_sb[:, jt * P:(jt + 1) * P],
                    rhs=wT_sb,
                    start=(jt == 0),
                    stop=False,
                )
                nc.tensor.matmul(
                    h_all[:, jt * H:(jt + 1) * H],
                    lhsT=ones_row,
                    rhs=bias_sb,
                    start=False,
                    stop=(jt == NJT - 1),
                )

            # m_rep[:, jt, r, :] = m[jt*128:(jt+1)*128, :] for every r (IC copies)
            m_rep = consts.tile([P, NJT, IC, H], F32)
            for jt in range(NJT):
                h_ps = h_all[:, jt * H:(jt + 1) * H]
                stats = small.tile([P, 6], F32)
                nc.vector.bn_stats(stats, h_ps)
                mv = small.tile([P, 2], F32)
                nc.vector.bn_aggr(mv, stats)
                xc = small.tile([P, H], F32)
                nc.vector.tensor_scalar_sub(xc, h_ps, mv[:, 0:1])
                stdv = small.tile([P, 1], F32)
                nc.scalar.activation(
                    stdv, mv[:, 1:2], mybir.ActivationFunctionType.Sqrt, bias=eps_t
                )
                rstd = small.tile([P, 1], F32)
                nc.vector.reciprocal(rstd, stdv)
                xn = small.tile([P, H], F32)
                nc.vector.tensor_scalar_mul(xn, xc, rstd)
                sg = small.tile([P, H], F32)
                nc.scalar.activation(sg, xn, mybir.ActivationFunctionType.Sigmoid)
                nc.vector.tensor_mul(m_rep[:, jt, 0, :], xn, sg)
                rep = 1
                while rep < IC:
                    cnt = min(rep, IC - rep)
                    nc.vector.tensor_copy(
                        m_rep[:, jt, rep:rep + cnt, :], m_rep[:, jt, 0:cnt, :]
                    )
                    rep += cnt

            # stage-1 PSUM pool stays open: releasing it would put release
            # waits on stage-2 Matmults, which walrus cannot encode.
            # ------------- out[i,h] = sum_j mask[j,i,h] * m[j,h] -------------
            # acc_c[k, f] += sel[:, 0:k+1].T-routed column sum of the product
            # tile over the j partition axis.  One accumulation group per PSUM
            # bank spans the whole loop -> no mid-loop release waits.
            opsum_cm = tc.tile_pool(name="opsum", bufs=1, space="PSUM")
            opsum = opsum_cm.__enter__()
            accs = [
                opsum.tile([P, MMF], F32, name=f"acc{c}", tag=f"acc{c}")
                for c in range(CH // MMF)
            ]
            for k in range(nch):
                mt = loads.tile([P, NJT * CH], F32)
                src = mask_d[:, k * IC:(k + 1) * IC, :].rearrange(
                    "(jt p) i h -> p jt i h", p=P
                )
                nc.sync.dma_start(
                    out=mt.rearrange("p (jt i h) -> p jt i h", jt=NJT, i=IC),
                    in_=src,
                )
                # in-place multiply: product overwrites the mask tile
                nc.vector.tensor_mul(
                    mt, mt, m_rep[:, :, :, :].rearrange("p a b c -> p (a b c)")
                )
                for jt in range(NJT):
                    for c in range(CH // MMF):
                        nc.tensor.matmul(
                            accs[c][0:nch, :],
                            lhsT=sel[:, k * nch:(k + 1) * nch],
                            rhs=mt[:, jt * CH + c * MMF:jt * CH + (c + 1) * MMF],
                            start=(k == 0 and jt == 0),
                            stop=(k == nch - 1 and jt == NJT - 1),
                        )
            # epilogue: PSUM partition k, column (i_loc, h) -> out row k*IC+i_loc
            o_sb = outs.tile([nch, CH], F32)
            for c in range(CH // MMF):
                nc.scalar.copy(o_sb[:, c * MMF:(c + 1) * MMF], accs[c][0:nch, :])
            nc.sync.dma_start(
                out=out_d[:, :].rearrange("(k i) h -> k (i h)", i=IC), in_=o_sb
            )
            opsum_cm.__exit__(None, None, None)
            stage1_psum.__exit__(None, None, None)
    nc.finalize()
    return nc


_NC_CACHE = {}


def _get_nc():
    key = "main"
    if key not in _NC_CACHE:
        _NC_CACHE[key] = build_nc()
    return _NC_CACHE[key]


def kernel(s, ef_mask, W, b):
    s = np.ascontiguousarray(s, dtype=np.float32)
    W = np.ascontiguousarray(W, dtype=np.float32)
    b = np.ascontiguousarray(b, dtype=np.float32)

    nc = _get_nc()
    in_maps = []
    for c in range(N_CORES):
        bb = c // 2
        half = c % 2
        shard = np.ascontiguousarray(
            np.asarray(
                ef_mask[bb, half * IH:(half + 1) * IH], dtype=np.float32
            ).transpose(1, 0, 2)
        )
        in_maps.append({"s": s[bb], "w": W, "b": b, "mask": shard})
    res = run_bass_kernel_spmd(nc, in_maps, list(range(N_CORES))).results
    out = np.empty((B, N, H), dtype=np.float32)
    for c in range(N_CORES):
        bb = c // 2
        half = c % 2
        out[bb, half * IH:(half + 1) * IH] = res[c]["out"]
    return out


# revision 10
# speedup vs baseline: 1.9510x; 1.1087x over previous
"""Trainium2 Bass kernel for DirCFConv-style GNN message passing.

Computes, for inputs s:(B,N,H) f32, ef_mask:(B,N,N,H) f32, W:(H,H), b:(H,):
    m   = SiLU(LayerNorm(s @ W.T + b))          # (B,N,H)
    out[b,i,h] = sum_j ef_mask[b,i,j,h] * m[b,j,h]

Sharding: 8 cores, core c handles batch b = c // 2 and query-node half
i in [ (c%2)*256, (c%2)*256+256 ).  The 64 MiB mask shard is laid out
[j, i, h] (transposed during host-side sharding) so every mask DMA is
128 partitions x 32 KiB fully-contiguous lines (~full HBM bandwidth).
Per i-chunk of 16, one 4 MiB DMA brings in all j; the vector engine
multiplies by a replicated m tile; the tensor engine column-sum-reduces
over the j partition axis with a ones-selector matmul whose output
partition k holds chunk k, accumulating all chunks in one PSUM group.
"""

import numpy as np

import concourse.bass as bass
import concourse.bacc as bacc
import concourse.tile as tile
from concourse import mybir
from concourse.bass_utils import run_bass_kernel_spmd
from concourse.masks import make_identity

B, N, H = 4, 512, 128
P = 128
NJT = N // P          # 4 j-tiles of 128 partitions
IC = 16               # i's per chunk -> 4 MiB DMAs, PSUM partition k = chunk k
IH = N // 2           # 256 i's per core
N_CORES = 8
LN_EPS = 1e-5
F32 = mybir.dt.float32
BF16 = mybir.dt.bfloat16
CH = IC * H           # 2048 columns per (jt, chunk)
MMF = 512             # moving-operand columns per matmul (one PSUM bank)


def build_nc(ih=IH):
    nc = bacc.Bacc()
    s_d = nc.declare_dram_parameter("s", [N, H], F32, isOutput=False)
    w_d = nc.declare_dram_parameter("w", [H, H], F32, isOutput=False)
    b_d = nc.declare_dram_parameter("b", [H], F32, isOutput=False)
    # mask is the [j, i, h]-transposed shard
    mask_d = nc.declare_dram_parameter("mask", [N, ih, H], F32, isOutput=False)
    out_d = nc.declare_dram_parameter("out", [ih, H], F32, isOutput=True)

    nch = ih // IC        # i-chunks; chunk k lands on PSUM partition k
    with tile.TileContext(nc) as tc:
        with (
            tc.tile_pool(name="consts", bufs=1) as consts,
            tc.tile_pool(name="small", bufs=4) as small,
            tc.tile_pool(name="loads", bufs=2) as loads,
            tc.tile_pool(name="prods", bufs=2) as prods,
            tc.tile_pool(name="outs", bufs=1) as outs,
        ):
            stage1_psum = tc.tile_pool(name="spsum", bufs=1, space="PSUM")
            spsum = stage1_psum.__enter__()
            # ---------------- constants ----------------
            # All constants are produced on gpsimd BEFORE make_identity so the
            # single carrier wait (Pool sem) covers every one of them.
            ones_row = consts.tile([1, P], F32)
            nc.gpsimd.memset(ones_row, 1.0)
            # sel[:, k*nch:(k+1)*nch] is a one-hot stationary operand routing
            # chunk k's column-sum to PSUM partition k (zeros to the others,
            # keeping every matmul's footprint the full [nch, MMF] region).
            sel = consts.tile([P, nch * nch], BF16)
            nc.gpsimd.memset(sel, 0.0)
            for k in range(nch):
                nc.gpsimd.memset(sel[:, k * nch + k:k * nch + k + 1], 1.0)
            # eps on DVE: its consumer (ACT Sqrt) already waits on DVE for mv,
            # and one DVE sem wait covers both (Activation also allows only 1).
            eps_t = consts.tile([P, 1], F32)
            nc.vector.memset(eps_t, LN_EPS)
            ident = consts.tile([P, P], F32)
            make_identity(nc, ident)

            w_sb = consts.tile([H, H], F32)
            nc.sync.dma_start(out=w_sb, in_=w_d[:, :])
            bias_sb = consts.tile([1, H], F32)
            b_ap = b_d[:]
            bias_src = bass.AP(
                tensor=b_ap.tensor, offset=b_ap.offset, ap=[[0, 1]] + list(b_ap.ap)
            )
            nc.sync.dma_start(out=bias_sb, in_=bias_src)

            # Wait-carrier: walrus allows only ONE sync wait per Matmult, so
            # absorb the gpsimd(memsets) dependency into a throwaway PE op;
            # later matmuls then only carry their own single DMA/engine wait.
            carrier_ps = spsum.tile([P, P], F32)
            nc.tensor.transpose(carrier_ps, ident, ident)

            # W^T via PE-transpose: (o,h) -> (h,o)
            wT_ps = spsum.tile([H, H], F32)
            nc.tensor.transpose(wT_ps, w_sb, ident)
            wT_sb = consts.tile([H, H], F32)
            nc.scalar.copy(wT_sb, wT_ps)

            # ------------- m = SiLU(LN(s @ W.T + b)) -------------
            # All four s^T blocks share one PSUM bank (one zero-region group);
            # likewise the four h = s@W.T+b blocks.  No PSUM slot rotation ->
            # no extra release waits on any Matmult.
            sT_all = spsum.tile([P, NJT * P], F32)
            h_all = spsum.tile([P, NJT * H], F32)
            s_sbs = []
            for jt in range(NJT):
                s_sb = small.tile([P, H], F32, tag=f"s_sb{jt}")
                nc.sync.dma_start(out=s_sb, in_=s_d[jt * P:(jt + 1) * P, :])
                s_sbs.append(s_sb)
                nc.tensor.matmul(
                    sT_all[:, jt * P:(jt + 1) * P],
                    lhsT=s_sb,
                    rhs=ident,
                    is_transpose=True,
                    start=(jt == 0),
                    stop=(jt == NJT - 1),
                )
            sT_sb = consts.tile([P, NJT * P], F32)
            nc.scalar.copy(sT_sb, sT_all)
            for jt in range(NJT):
                nc.tensor.matmul(
                    h_all[:, jt * H:(jt + 1) * H],
                    lhsT=sT_sb[:, jt * P:(jt + 1) * P],
                    rhs=wT_sb,
                    start=(jt == 0),
                    stop=False,
                )
                nc.tensor.matmul(
                    h_all[:, jt * H:(jt + 1) * H],
                    lhsT=ones_row,
                    rhs=bias_sb,
                    start=False,
                    stop=(jt == NJT - 1),
                )

            # m_rep[:, jt, r, :] = m[jt*128:(jt+1)*128, :] for every r (IC copies)
            m_rep = consts.tile([P, NJT, IC, H], F32)
            for jt in range(NJT):
                h_ps = h_all[:, jt * H:(jt + 1) * H]
                stats = small.tile([P, 6], F32)
                nc.vector.bn_stats(stats, h_ps)
                mv = small.tile([P, 2], F32)
                nc.vector.bn_aggr(mv, stats)
                xc = small.tile([P, H], F32)
                nc.vector.tensor_scalar_sub(xc, h_ps, mv[:, 0:1])
                stdv = small.tile([P, 1], F32)
                nc.scalar.activation(
                    stdv, mv[:, 1:2], mybir.ActivationFunctionType.Sqrt, bias=eps_t
                )
                rstd = small.tile([P, 1], F32)
                nc.vector.reciprocal(rstd, stdv)
                xn = small.tile([P, H], F32)
                nc.vector.tensor_scalar_mul(xn, xc, rstd)
                sg = small.tile([P, H], F32)
                nc.scalar.activation(sg, xn, mybir.ActivationFunctionType.Sigmoid)
                nc.vector.tensor_mul(m_rep[:, jt, 0, :], xn, sg)
                rep = 1
                while rep < IC:
                    cnt = min(rep, IC - rep)
                    nc.vector.tensor_copy(
                        m_rep[:, jt, rep:rep + cnt, :], m_rep[:, jt, 0:cnt, :]
                    )
                    rep += cnt

            # stage-1 PSUM pool stays open: releasing it would put release
            # waits on stage-2 Matmults, which walrus cannot encode.
            # ------------- out[i,h] = sum_j mask[j,i,h] * m[j,h] -------------
            # acc_c[k, f] += sel[:, 0:k+1].T-routed column sum of the product
            # tile over the j partition axis.  One accumulation group per PSUM
            # bank spans the whole loop -> no mid-loop release waits.
            opsum_cm = tc.tile_pool(name="opsum", bufs=1, space="PSUM")
            opsum = opsum_cm.__enter__()
            accs = [
                opsum.tile([P, MMF], F32, name=f"acc{c}", tag=f"acc{c}")
                for c in range(CH // MMF)
            ]
            for k in range(nch):
                mt = loads.tile([P, NJT * CH], F32)
                src = mask_d[:, k * IC:(k + 1) * IC, :].rearrange(
                    "(jt p) i h -> p jt i h", p=P
                )
                nc.sync.dma_start(
                    out=mt.rearrange("p (jt i h) -> p jt i h", jt=NJT, i=IC),
                    in_=src,
                )
                # product in bf16: fp32 moving operands stream the PE at 1/4
                # rate, bf16 at full rate; DVE computes fp32 internally and
                # PSUM accumulation stays fp32.
                pt = prods.tile([P, NJT * CH], BF16)
                nc.vector.tensor_mul(
                    pt, mt, m_rep[:, :, :, :].rearrange("p a b c -> p (a b c)")
                )
                for jt in range(NJT):
                    for c in range(CH // MMF):
                        nc.tensor.matmul(
                            accs[c][0:nch, :],
                            lhsT=sel[:, k * nch:(k + 1) * nch],
                            rhs=pt[:, jt * CH + c * MMF:jt * CH + (c + 1) * MMF],
                            start=(k == 0 and jt == 0),
                            stop=(k == nch - 1 and jt == NJT - 1),
                        )
            # epilogue: PSUM partition k, column (i_loc, h) -> out row k*IC+i_loc
            o_sb = outs.tile([nch, CH], F32)
            for c in range(CH // MMF):
                nc.scalar.copy(o_sb[:, c * MMF:(c + 1) * MMF], accs[c][0:nch, :])
            nc.sync.dma_start(
                out=out_d[:, :].rearrange("(k i) h -> k (i h)", i=IC), in_=o_sb
            )
            opsum_cm.__exit__(None, None, None)
            stage1_psum.__exit__(None, None, None)
    nc.finalize()
    return nc


_NC_CACHE = {}


def _get_nc():
    key = "main"
    if key not in _NC_CACHE:
        _NC_CACHE[key] = build_nc()
    return _NC_CACHE[key]


def kernel(s, ef_mask, W, b):
    s = np.ascontiguousarray(s, dtype=np.float32)
    W = np.ascontiguousarray(W, dtype=np.float32)
    b = np.ascontiguousarray(b, dtype=np.float32)

    nc = _get_nc()
    in_maps = []
    for c in range(N_CORES):
        bb = c // 2
        half = c % 2
        shard = np.ascontiguousarray(
            np.asarray(
                ef_mask[bb, half * IH:(half + 1) * IH], dtype=np.float32
            ).transpose(1, 0, 2)
        )
        in_maps.append({"s": s[bb], "w": W, "b": b, "mask": shard})
    res = run_bass_kernel_spmd(nc, in_maps, list(range(N_CORES))).results
    out = np.empty((B, N, H), dtype=np.float32)
    for c in range(N_CORES):
        bb = c // 2
        half = c % 2
        out[bb, half * IH:(half + 1) * IH] = res[c]["out"]
    return out


# revision 12
# speedup vs baseline: 2.0280x; 1.0394x over previous
"""Trainium2 Bass kernel for DirCFConv-style GNN message passing.

Computes, for inputs s:(B,N,H) f32, ef_mask:(B,N,N,H) f32, W:(H,H), b:(H,):
    m   = SiLU(LayerNorm(s @ W.T + b))          # (B,N,H)
    out[b,i,h] = sum_j ef_mask[b,i,j,h] * m[b,j,h]

Sharding: 8 cores, core c handles batch b = c // 2 and query-node half
i in [ (c%2)*256, (c%2)*256+256 ).  The 64 MiB mask shard is laid out
[j, i, h] (transposed during host-side sharding) so every mask DMA is
128 partitions x 32 KiB fully-contiguous lines (~full HBM bandwidth).
Per i-chunk of 16, one 4 MiB DMA brings in all j; the vector engine
multiplies by a replicated m tile; the tensor engine column-sum-reduces
over the j partition axis with a ones-selector matmul whose output
partition k holds chunk k, accumulating all chunks in one PSUM group.
"""

import numpy as np

import concourse.bass as bass
import concourse.bacc as bacc
import concourse.tile as tile
from concourse import mybir
from concourse.bass_utils import run_bass_kernel_spmd
from concourse.masks import make_identity

B, N, H = 4, 512, 128
P = 128
NJT = N // P          # 4 j-tiles of 128 partitions
IC = 16               # i's per chunk -> 4 MiB DMAs, PSUM partition k = chunk k
IH = N // 2           # 256 i's per core
N_CORES = 8
LN_EPS = 1e-5
F32 = mybir.dt.float32
BF16 = mybir.dt.bfloat16
CH = IC * H           # 2048 columns per (jt, chunk)
MMF = 512             # moving-operand columns per matmul (one PSUM bank)


def build_nc(ih=IH):
    nc = bacc.Bacc()
    s_d = nc.declare_dram_parameter("s", [N, H], F32, isOutput=False)
    w_d = nc.declare_dram_parameter("w", [H, H], F32, isOutput=False)
    b_d = nc.declare_dram_parameter("b", [H], F32, isOutput=False)
    # mask is the [j, i, h]-transposed shard
    mask_d = nc.declare_dram_parameter("mask", [N, ih, H], F32, isOutput=False)
    out_d = nc.declare_dram_parameter("out", [ih, H], F32, isOutput=True)

    nch = ih // IC        # i-chunks; chunk k lands on PSUM partition k
    with tile.TileContext(nc) as tc:
        with (
            tc.tile_pool(name="consts", bufs=1) as consts,
            tc.tile_pool(name="small", bufs=4) as small,
            tc.tile_pool(name="loads", bufs=3) as loads,
            tc.tile_pool(name="prods", bufs=2) as prods,
            tc.tile_pool(name="outs", bufs=1) as outs,
        ):
            stage1_psum = tc.tile_pool(name="spsum", bufs=1, space="PSUM")
            spsum = stage1_psum.__enter__()
            # ---------------- constants ----------------
            # All constants are produced on gpsimd BEFORE make_identity so the
            # single carrier wait (Pool sem) covers every one of them.
            ones_row = consts.tile([1, P], F32)
            nc.gpsimd.memset(ones_row, 1.0)
            # sel[:, k*nch:(k+1)*nch] is a one-hot stationary operand routing
            # chunk k's column-sum to PSUM partition k (zeros to the others,
            # keeping every matmul's footprint the full [nch, MMF] region).
            sel = consts.tile([P, nch * nch], BF16)
            nc.gpsimd.memset(sel, 0.0)
            for k in range(nch):
                nc.gpsimd.memset(sel[:, k * nch + k:k * nch + k + 1], 1.0)
            # eps on DVE: its consumer (ACT Sqrt) already waits on DVE for mv,
            # and one DVE sem wait covers both (Activation also allows only 1).
            eps_t = consts.tile([P, 1], F32)
            nc.vector.memset(eps_t, LN_EPS)
            ident = consts.tile([P, P], F32)
            make_identity(nc, ident)

            w_sb = consts.tile([H, H], F32)
            nc.sync.dma_start(out=w_sb, in_=w_d[:, :])
            bias_sb = consts.tile([1, H], F32)
            b_ap = b_d[:]
            bias_src = bass.AP(
                tensor=b_ap.tensor, offset=b_ap.offset, ap=[[0, 1]] + list(b_ap.ap)
            )
            nc.sync.dma_start(out=bias_sb, in_=bias_src)

            # Wait-carrier: walrus allows only ONE sync wait per Matmult, so
            # absorb the gpsimd(memsets) dependency into a throwaway PE op;
            # later matmuls then only carry their own single DMA/engine wait.
            carrier_ps = spsum.tile([P, P], F32)
            nc.tensor.transpose(carrier_ps, ident, ident)

            # W^T via PE-transpose: (o,h) -> (h,o)
            wT_ps = spsum.tile([H, H], F32)
            nc.tensor.transpose(wT_ps, w_sb, ident)
            wT_sb = consts.tile([H, H], F32)
            nc.scalar.copy(wT_sb, wT_ps)

            # ------------- m = SiLU(LN(s @ W.T + b)) -------------
            # All four s^T blocks share one PSUM bank (one zero-region group);
            # likewise the four h = s@W.T+b blocks.  No PSUM slot rotation ->
            # no extra release waits on any Matmult.
            sT_all = spsum.tile([P, NJT * P], F32)
            h_all = spsum.tile([P, NJT * H], F32)
            s_sbs = []
            for jt in range(NJT):
                s_sb = small.tile([P, H], F32, tag=f"s_sb{jt}")
                nc.sync.dma_start(out=s_sb, in_=s_d[jt * P:(jt + 1) * P, :])
                s_sbs.append(s_sb)
                nc.tensor.matmul(
                    sT_all[:, jt * P:(jt + 1) * P],
                    lhsT=s_sb,
                    rhs=ident,
                    is_transpose=True,
                    start=(jt == 0),
                    stop=(jt == NJT - 1),
                )
            sT_sb = consts.tile([P, NJT * P], F32)
            nc.scalar.copy(sT_sb, sT_all)
            for jt in range(NJT):
                nc.tensor.matmul(
                    h_all[:, jt * H:(jt + 1) * H],
                    lhsT=sT_sb[:, jt * P:(jt + 1) * P],
                    rhs=wT_sb,
                    start=(jt == 0),
                    stop=False,
                )
                nc.tensor.matmul(
                    h_all[:, jt * H:(jt + 1) * H],
                    lhsT=ones_row,
                    rhs=bias_sb,
                    start=False,
                    stop=(jt == NJT - 1),
                )

            # m_rep[:, jt, r, :] = m[jt*128:(jt+1)*128, :] for every r (IC copies)
            m_rep = consts.tile([P, NJT, IC, H], F32)
            for jt in range(NJT):
                h_ps = h_all[:, jt * H:(jt + 1) * H]
                stats = small.tile([P, 6], F32)
                nc.vector.bn_stats(stats, h_ps)
                mv = small.tile([P, 2], F32)
                nc.vector.bn_aggr(mv, stats)
                xc = small.tile([P, H], F32)
                nc.vector.tensor_scalar_sub(xc, h_ps, mv[:, 0:1])
                stdv = small.tile([P, 1], F32)
                nc.scalar.activation(
                    stdv, mv[:, 1:2], mybir.ActivationFunctionType.Sqrt, bias=eps_t
                )
                rstd = small.tile([P, 1], F32)
                nc.vector.reciprocal(rstd, stdv)
                xn = small.tile([P, H], F32)
                nc.vector.tensor_scalar_mul(xn, xc, rstd)
                sg = small.tile([P, H], F32)
                nc.scalar.activation(sg, xn, mybir.ActivationFunctionType.Sigmoid)
                nc.vector.tensor_mul(m_rep[:, jt, 0, :], xn, sg)
                rep = 1
                while rep < IC:
                    cnt = min(rep, IC - rep)
                    nc.vector.tensor_copy(
                        m_rep[:, jt, rep:rep + cnt, :], m_rep[:, jt, 0:cnt, :]
                    )
                    rep += cnt

            # stage-1 PSUM pool stays open: releasing it would put release
            # waits on stage-2 Matmults, which walrus cannot encode.
            # ------------- out[i,h] = sum_j mask[j,i,h] * m[j,h] -------------
            # acc_c[k, f] += sel[:, 0:k+1].T-routed column sum of the product
            # tile over the j partition axis.  One accumulation group per PSUM
            # bank spans the whole loop -> no mid-loop release waits.
            opsum_cm = tc.tile_pool(name="opsum", bufs=1, space="PSUM")
            opsum = opsum_cm.__enter__()
            accs = [
                opsum.tile([P, MMF], F32, name=f"acc{c}", tag=f"acc{c}")
                for c in range(CH // MMF)
            ]
            for k in range(nch):
                mt = loads.tile([P, NJT * CH], F32)
                src = mask_d[:, k * IC:(k + 1) * IC, :].rearrange(
                    "(jt p) i h -> p jt i h", p=P
                )
                nc.sync.dma_start(
                    out=mt.rearrange("p (jt i h) -> p jt i h", jt=NJT, i=IC),
                    in_=src,
                )
                # product in bf16: fp32 moving operands stream the PE at 1/4
                # rate, bf16 at full rate; DVE computes fp32 internally and
                # PSUM accumulation stays fp32.
                pt = prods.tile([P, NJT * CH], BF16)
                for jt in range(NJT):
                    # per-jt multiply: finer DVE->PE pipelining, and PE gets
                    # work every ~2us which keeps the HAM clock-gate warm
                    nc.vector.tensor_mul(
                        pt[:, jt * CH:(jt + 1) * CH],
                        mt[:, jt * CH:(jt + 1) * CH],
                        m_rep[:, jt, :, :].rearrange("p a b -> p (a b)"),
                    )
                    for c in range(CH // MMF):
                        nc.tensor.matmul(
                            accs[c][0:nch, :],
                            lhsT=sel[:, k * nch:(k + 1) * nch],
                            rhs=pt[:, jt * CH + c * MMF:jt * CH + (c + 1) * MMF],
                            start=(k == 0 and jt == 0),
                            stop=(k == nch - 1 and jt == NJT - 1),
                        )
            # epilogue: PSUM partition k, column (i_loc, h) -> out row k*IC+i_loc
            o_sb = outs.tile([nch, CH], F32)
            for c in range(CH // MMF):
                nc.scalar.copy(o_sb[:, c * MMF:(c + 1) * MMF], accs[c][0:nch, :])
            nc.sync.dma_start(
                out=out_d[:, :].rearrange("(k i) h -> k (i h)", i=IC), in_=o_sb
            )
            opsum_cm.__exit__(None, None, None)
            stage1_psum.__exit__(None, None, None)
    nc.finalize()
    return nc


_NC_CACHE = {}


def _get_nc():
    key = "main"
    if key not in _NC_CACHE:
        _NC_CACHE[key] = build_nc()
    return _NC_CACHE[key]


def kernel(s, ef_mask, W, b):
    s = np.ascontiguousarray(s, dtype=np.float32)
    W = np.ascontiguousarray(W, dtype=np.float32)
    b = np.ascontiguousarray(b, dtype=np.float32)

    nc = _get_nc()
    in_maps = []
    for c in range(N_CORES):
        bb = c // 2
        half = c % 2
        shard = np.ascontiguousarray(
            np.asarray(
                ef_mask[bb, half * IH:(half + 1) * IH], dtype=np.float32
            ).transpose(1, 0, 2)
        )
        in_maps.append({"s": s[bb], "w": W, "b": b, "mask": shard})
    res = run_bass_kernel_spmd(nc, in_maps, list(range(N_CORES))).results
    out = np.empty((B, N, H), dtype=np.float32)
    for c in range(N_CORES):
        bb = c // 2
        half = c % 2
        out[bb, half * IH:(half + 1) * IH] = res[c]["out"]
    return out


# revision 17
# speedup vs baseline: 2.1062x; 1.0386x over previous
"""Trainium2 Bass kernel for DirCFConv-style GNN message passing.

Computes, for inputs s:(B,N,H) f32, ef_mask:(B,N,N,H) f32, W:(H,H), b:(H,):
    m   = SiLU(LayerNorm(s @ W.T + b))          # (B,N,H)
    out[b,i,h] = sum_j ef_mask[b,i,j,h] * m[b,j,h]

Sharding: 8 cores, core c handles batch b = c // 2 and query-node half
i in [ (c%2)*256, (c%2)*256+256 ).  The 64 MiB mask shard is laid out
[j, i, h] (transposed during host-side sharding) so every mask DMA is
128 partitions x 32 KiB fully-contiguous lines (~full HBM bandwidth).
Per i-chunk of 16, one 4 MiB DMA brings in all j; the vector engine
multiplies by a replicated m tile; the tensor engine column-sum-reduces
over the j partition axis with a ones-selector matmul whose output
partition k holds chunk k, accumulating all chunks in one PSUM group.
"""

import numpy as np

import concourse.bass as bass
import concourse.bacc as bacc
import concourse.tile as tile
from concourse import mybir
from concourse.bass_utils import run_bass_kernel_spmd
from concourse.masks import make_identity

B, N, H = 4, 512, 128
P = 128
NJT = N // P          # 4 j-tiles of 128 partitions
IC = 16               # i's per chunk -> 4 MiB DMAs, PSUM partition k = chunk k
IH = N // 2           # 256 i's per core
N_CORES = 8
LN_EPS = 1e-5
F32 = mybir.dt.float32
BF16 = mybir.dt.bfloat16
CH = IC * H           # 2048 columns per (jt, chunk)
MMF = 512             # moving-operand columns per matmul (one PSUM bank)


def build_nc(ih=IH):
    nc = bacc.Bacc()
    s_d = nc.declare_dram_parameter("s", [N, H], F32, isOutput=False)
    w_d = nc.declare_dram_parameter("w", [H, H], F32, isOutput=False)
    b_d = nc.declare_dram_parameter("b", [H], F32, isOutput=False)
    nch = ih // IC        # i-chunks; chunk k lands on PSUM partition k
    # mask shard pre-laid [k, jt, p, ic*h] so every (k, jt) DMA is one fully
    # sequential 1 MiB HBM read into 128 partitions x 8 KiB lines
    mask_d = nc.declare_dram_parameter(
        "mask", [nch, NJT, P, IC * H], F32, isOutput=False
    )
    out_d = nc.declare_dram_parameter("out", [ih, H], F32, isOutput=True)

    with tile.TileContext(nc) as tc:
        with (
            tc.tile_pool(name="consts", bufs=1) as consts,
            tc.tile_pool(name="small", bufs=4) as small,
            tc.tile_pool(name="loads", bufs=3) as loads,
            tc.tile_pool(name="prods", bufs=2) as prods,
            tc.tile_pool(name="outs", bufs=1) as outs,
        ):
            stage1_psum = tc.tile_pool(name="spsum", bufs=1, space="PSUM")
            spsum = stage1_psum.__enter__()
            # ---------------- constants ----------------
            # All constants are produced on gpsimd BEFORE make_identity so the
            # single carrier wait (Pool sem) covers every one of them.
            ones_row = consts.tile([1, P], F32)
            nc.gpsimd.memset(ones_row, 1.0)
            # sel[:, k*nch:(k+1)*nch] is a one-hot stationary operand routing
            # chunk k's column-sum to PSUM partition k (zeros to the others,
            # keeping every matmul's footprint the full [nch, MMF] region).
            sel = consts.tile([P, nch * nch], BF16)
            nc.gpsimd.memset(sel, 0.0)
            for k in range(nch):
                nc.gpsimd.memset(sel[:, k * nch + k:k * nch + k + 1], 1.0)
            # eps on DVE: its consumer (ACT Sqrt) already waits on DVE for mv,
            # and one DVE sem wait covers both (Activation also allows only 1).
            eps_t = consts.tile([P, 1], F32)
            nc.vector.memset(eps_t, LN_EPS)
            ident = consts.tile([P, P], F32)
            make_identity(nc, ident)

            w_sb = consts.tile([H, H], F32)
            nc.scalar.dma_start(out=w_sb, in_=w_d[:, :])
            bias_sb = consts.tile([1, H], F32)
            b_ap = b_d[:]
            bias_src = bass.AP(
                tensor=b_ap.tensor, offset=b_ap.offset, ap=[[0, 1]] + list(b_ap.ap)
            )
            nc.scalar.dma_start(out=bias_sb, in_=bias_src)

            # Wait-carrier: walrus allows only ONE sync wait per Matmult, so
            # absorb the gpsimd(memsets) dependency into a throwaway PE op;
            # later matmuls then only carry their own single DMA/engine wait.
            carrier_ps = spsum.tile([P, P], F32)
            nc.tensor.transpose(carrier_ps, ident, ident)

            # Pre-issue the first chunks' mask DMAs on the sync queue so the
            # 64 MiB stream starts immediately; stage-1's small loads go via
            # the scalar HWDGE queue and interleave at packet granularity.
            def issue_mask_load(k):
                mt = loads.tile([P, NJT * CH], F32, name=f"mt{k}", tag="mt")
                for jt in range(NJT):
                    nc.sync.dma_start(
                        out=mt[:, jt * CH:(jt + 1) * CH], in_=mask_d[k, jt]
                    )
                return mt

            nbuf = min(3, nch)
            pre_mts = [issue_mask_load(k) for k in range(nbuf)]

            # W^T via PE-transpose: (o,h) -> (h,o)
            wT_ps = spsum.tile([H, H], F32)
            nc.tensor.transpose(wT_ps, w_sb, ident)
            wT_sb = consts.tile([H, H], F32)
            nc.scalar.copy(wT_sb, wT_ps)

            # ------------- m = SiLU(LN(s @ W.T + b)) -------------
            # All four s^T blocks share one PSUM bank (one zero-region group);
            # likewise the four h = s@W.T+b blocks.  No PSUM slot rotation ->
            # no extra release waits on any Matmult.
            sT_all = spsum.tile([P, NJT * P], F32)
            h_all = spsum.tile([P, NJT * H], F32)
            s_sbs = []
            for jt in range(NJT):
                s_sb = small.tile([P, H], F32, tag=f"s_sb{jt}")
                nc.scalar.dma_start(out=s_sb, in_=s_d[jt * P:(jt + 1) * P, :])
                s_sbs.append(s_sb)
                nc.tensor.matmul(
                    sT_all[:, jt * P:(jt + 1) * P],
                    lhsT=s_sb,
                    rhs=ident,
                    is_transpose=True,
                    start=(jt == 0),
                    stop=(jt == NJT - 1),
                )
            sT_sb = consts.tile([P, NJT * P], F32)
            nc.scalar.copy(sT_sb, sT_all)
            for jt in range(NJT):
                nc.tensor.matmul(
                    h_all[:, jt * H:(jt + 1) * H],
                    lhsT=sT_sb[:, jt * P:(jt + 1) * P],
                    rhs=wT_sb,
                    start=(jt == 0),
                    stop=False,
                )
                nc.tensor.matmul(
                    h_all[:, jt * H:(jt + 1) * H],
                    lhsT=ones_row,
                    rhs=bias_sb,
                    start=False,
                    stop=(jt == NJT - 1),
                )

            # m_rep[:, jt, r, :] = m[jt*128:(jt+1)*128, :] for every r (IC copies)
            m_rep = consts.tile([P, NJT, IC, H], F32)
            for jt in range(NJT):
                h_ps = h_all[:, jt * H:(jt + 1) * H]
                stats = small.tile([P, 6], F32)
                nc.vector.bn_stats(stats, h_ps)
                mv = small.tile([P, 2], F32)
                nc.vector.bn_aggr(mv, stats)
                xc = small.tile([P, H], F32)
                nc.vector.tensor_scalar_sub(xc, h_ps, mv[:, 0:1])
                stdv = small.tile([P, 1], F32)
                nc.scalar.activation(
                    stdv, mv[:, 1:2], mybir.ActivationFunctionType.Sqrt, bias=eps_t
                )
                rstd = small.tile([P, 1], F32)
                nc.vector.reciprocal(rstd, stdv)
                xn = small.tile([P, H], F32)
                nc.vector.tensor_scalar_mul(xn, xc, rstd)
                sg = small.tile([P, H], F32)
                nc.scalar.activation(sg, xn, mybir.ActivationFunctionType.Sigmoid)
                nc.vector.tensor_mul(m_rep[:, jt, 0, :], xn, sg)
                rep = 1
                while rep < IC:
                    cnt = min(rep, IC - rep)
                    nc.vector.tensor_copy(
                        m_rep[:, jt, rep:rep + cnt, :], m_rep[:, jt, 0:cnt, :]
                    )
                    rep += cnt

            # stage-1 PSUM pool stays open: releasing it would put release
            # waits on stage-2 Matmults, which walrus cannot encode.
            # ------------- out[i,h] = sum_j mask[j,i,h] * m[j,h] -------------
            # acc_c[k, f] += sel[:, 0:k+1].T-routed column sum of the product
            # tile over the j partition axis.  One accumulation group per PSUM
            # bank spans the whole loop -> no mid-loop release waits.
            opsum_cm = tc.tile_pool(name="opsum", bufs=1, space="PSUM")
            opsum = opsum_cm.__enter__()
            accs = [
                opsum.tile([P, MMF], F32, name=f"acc{c}", tag=f"acc{c}")
                for c in range(CH // MMF)
            ]
            for k in range(nch):
                mt = pre_mts[k] if k < nbuf else issue_mask_load(k)
                # product in bf16: fp32 moving operands stream the PE at 1/4
                # rate, bf16 at full rate; DVE computes fp32 internally and
                # PSUM accumulation stays fp32.
                pt = prods.tile([P, NJT * CH], BF16)
                for jt in range(NJT):
                    # per-jt multiply: finer DVE->PE pipelining, and PE gets
                    # work every ~2us which keeps the HAM clock-gate warm
                    nc.vector.tensor_mul(
                        pt[:, jt * CH:(jt + 1) * CH],
                        mt[:, jt * CH:(jt + 1) * CH],
                        m_rep[:, jt, :, :].rearrange("p a b -> p (a b)"),
                    )
                    for c in range(CH // MMF):
                        nc.tensor.matmul(
                            accs[c][0:nch, :],
                            lhsT=sel[:, k * nch:(k + 1) * nch],
                            rhs=pt[:, jt * CH + c * MMF:jt * CH + (c + 1) * MMF],
                            start=(k == 0 and jt == 0),
                            stop=(k == nch - 1 and jt == NJT - 1),
                        )
            # epilogue: PSUM partition k, column (i_loc, h) -> out row k*IC+i_loc
            o_sb = outs.tile([nch, CH], F32)
            for c in range(CH // MMF):
                nc.scalar.copy(o_sb[:, c * MMF:(c + 1) * MMF], accs[c][0:nch, :])
            nc.sync.dma_start(
                out=out_d[:, :].rearrange("(k i) h -> k (i h)", i=IC), in_=o_sb
            )
            opsum_cm.__exit__(None, None, None)
            stage1_psum.__exit__(None, None, None)
    nc.finalize()
    return nc


_NC_CACHE = {}


def _get_nc():
    key = "main"
    if key not in _NC_CACHE:
        _NC_CACHE[key] = build_nc()
    return _NC_CACHE[key]


def kernel(s, ef_mask, W, b):
    s = np.ascontiguousarray(s, dtype=np.float32)
    W = np.ascontiguousarray(W, dtype=np.float32)
    b = np.ascontiguousarray(b, dtype=np.float32)

    nc = _get_nc()
    in_maps = []
    for c in range(N_CORES):
        bb = c // 2
        half = c % 2
        # [i, j, h] -> [k, jt, p, ic*h]: chunk-k/jt tiles are sequential in
        # HBM, partition p = j % 128, 8 KiB contiguous per-partition lines
        shard = (
            np.asarray(ef_mask[bb, half * IH:(half + 1) * IH], dtype=np.float32)
            .transpose(1, 0, 2)                    # [j, i, h]
            .reshape(NJT, P, IH // IC, IC, H)      # [jt, p, k, ic, h]
            .transpose(2, 0, 1, 3, 4)              # [k, jt, p, ic, h]
            .reshape(IH // IC, NJT, P, IC * H)
        )
        in_maps.append(
            {"s": s[bb], "w": W, "b": b, "mask": np.ascontiguousarray(shard)}
        )
    res = run_bass_kernel_spmd(nc, in_maps, list(range(N_CORES))).results
    out = np.empty((B, N, H), dtype=np.float32)
    for c in range(N_CORES):
        bb = c // 2
        half = c % 2
        out[bb, half * IH:(half + 1) * IH] = res[c]["out"]
    return out


# revision 19
# speedup vs baseline: 3.7608x; 1.7856x over previous
"""Trainium2 Bass kernel for DirCFConv-style GNN message passing.

Computes, for inputs s:(B,N,H) f32, ef_mask:(B,N,N,H) f32, W:(H,H), b:(H,):
    m   = SiLU(LayerNorm(s @ W.T + b))          # (B,N,H)
    out[b,i,h] = sum_j ef_mask[b,i,j,h] * m[b,j,h]

Sharding: 8 cores, core c handles batch b = c // 2 and query-node half
i in [ (c%2)*256, (c%2)*256+256 ).  The 64 MiB mask shard is laid out
[j, i, h] (transposed during host-side sharding) so every mask DMA is
128 partitions x 32 KiB fully-contiguous lines (~full HBM bandwidth).
Per i-chunk of 16, one 4 MiB DMA brings in all j; the vector engine
multiplies by a replicated m tile; the tensor engine column-sum-reduces
over the j partition axis with a ones-selector matmul whose output
partition k holds chunk k, accumulating all chunks in one PSUM group.
"""

import numpy as np

import concourse.bass as bass
import concourse.bacc as bacc
import concourse.tile as tile
from concourse import mybir
from concourse.bass_utils import run_bass_kernel_spmd
from concourse.masks import make_identity

B, N, H = 4, 512, 128
P = 128
NJT = N // P          # 4 j-tiles of 128 partitions
IC = 16               # i's per chunk -> 4 MiB DMAs, PSUM partition k = chunk k
IH = N // 2           # 256 i's per core
N_CORES = 8
LN_EPS = 1e-5
F32 = mybir.dt.float32
BF16 = mybir.dt.bfloat16
F16 = mybir.dt.float16
CH = IC * H           # 2048 columns per (jt, chunk)
MMF = 512             # moving-operand columns per matmul (one PSUM bank)


def build_nc(ih=IH):
    nc = bacc.Bacc()
    s_d = nc.declare_dram_parameter("s", [N, H], F32, isOutput=False)
    w_d = nc.declare_dram_parameter("w", [H, H], F32, isOutput=False)
    b_d = nc.declare_dram_parameter("b", [H], F32, isOutput=False)
    nch = ih // IC        # i-chunks; chunk k lands on PSUM partition k
    # fp16 mask shard pre-laid [k, p, jt*ic*h]: half the HBM traffic, and
    # every chunk DMA is one fully sequential 2 MiB read into 128
    # partitions x 16 KiB lines
    mask_d = nc.declare_dram_parameter(
        "mask", [nch, P, NJT * IC * H], F16, isOutput=False
    )
    out_d = nc.declare_dram_parameter("out", [ih, H], F32, isOutput=True)

    with tile.TileContext(nc) as tc:
        with (
            tc.tile_pool(name="consts", bufs=1) as consts,
            tc.tile_pool(name="small", bufs=4) as small,
            tc.tile_pool(name="loads", bufs=6) as loads,
            tc.tile_pool(name="prods", bufs=2) as prods,
            tc.tile_pool(name="outs", bufs=1) as outs,
        ):
            stage1_psum = tc.tile_pool(name="spsum", bufs=1, space="PSUM")
            spsum = stage1_psum.__enter__()
            # ---------------- constants ----------------
            # All constants are produced on gpsimd BEFORE make_identity so the
            # single carrier wait (Pool sem) covers every one of them.
            ones_row = consts.tile([1, P], F32)
            nc.gpsimd.memset(ones_row, 1.0)
            # sel[:, k*nch:(k+1)*nch] is a one-hot stationary operand routing
            # chunk k's column-sum to PSUM partition k (zeros to the others,
            # keeping every matmul's footprint the full [nch, MMF] region).
            sel = consts.tile([P, nch * nch], F16)
            nc.gpsimd.memset(sel, 0.0)
            for k in range(nch):
                nc.gpsimd.memset(sel[:, k * nch + k:k * nch + k + 1], 1.0)
            # eps on DVE: its consumer (ACT Sqrt) already waits on DVE for mv,
            # and one DVE sem wait covers both (Activation also allows only 1).
            eps_t = consts.tile([P, 1], F32)
            nc.vector.memset(eps_t, LN_EPS)
            ident = consts.tile([P, P], F32)
            make_identity(nc, ident)

            w_sb = consts.tile([H, H], F32)
            nc.scalar.dma_start(out=w_sb, in_=w_d[:, :])
            bias_sb = consts.tile([1, H], F32)
            b_ap = b_d[:]
            bias_src = bass.AP(
                tensor=b_ap.tensor, offset=b_ap.offset, ap=[[0, 1]] + list(b_ap.ap)
            )
            nc.scalar.dma_start(out=bias_sb, in_=bias_src)

            # Wait-carrier: walrus allows only ONE sync wait per Matmult, so
            # absorb the gpsimd(memsets) dependency into a throwaway PE op;
            # later matmuls then only carry their own single DMA/engine wait.
            carrier_ps = spsum.tile([P, P], F32)
            nc.tensor.transpose(carrier_ps, ident, ident)

            # Pre-issue the first chunks' mask DMAs on the sync queue so the
            # 64 MiB stream starts immediately; stage-1's small loads go via
            # the scalar HWDGE queue and interleave at packet granularity.
            def issue_mask_load(k):
                mt = loads.tile([P, NJT * CH], F16, name=f"mt{k}", tag="mt")
                nc.sync.dma_start(out=mt, in_=mask_d[k])
                return mt

            nbuf = min(6, nch)
            pre_mts = [issue_mask_load(k) for k in range(nbuf)]

            # W^T via PE-transpose: (o,h) -> (h,o)
            wT_ps = spsum.tile([H, H], F32)
            nc.tensor.transpose(wT_ps, w_sb, ident)
            wT_sb = consts.tile([H, H], F32)
            nc.scalar.copy(wT_sb, wT_ps)

            # ------------- m = SiLU(LN(s @ W.T + b)) -------------
            # All four s^T blocks share one PSUM bank (one zero-region group);
            # likewise the four h = s@W.T+b blocks.  No PSUM slot rotation ->
            # no extra release waits on any Matmult.
            sT_all = spsum.tile([P, NJT * P], F32)
            h_all = spsum.tile([P, NJT * H], F32)
            s_sbs = []
            for jt in range(NJT):
                s_sb = small.tile([P, H], F32, tag=f"s_sb{jt}")
                nc.scalar.dma_start(out=s_sb, in_=s_d[jt * P:(jt + 1) * P, :])
                s_sbs.append(s_sb)
                nc.tensor.matmul(
                    sT_all[:, jt * P:(jt + 1) * P],
                    lhsT=s_sb,
                    rhs=ident,
                    is_transpose=True,
                    start=(jt == 0),
                    stop=(jt == NJT - 1),
                )
            sT_sb = consts.tile([P, NJT * P], F32)
            nc.scalar.copy(sT_sb, sT_all)
            for jt in range(NJT):
                nc.tensor.matmul(
                    h_all[:, jt * H:(jt + 1) * H],
                    lhsT=sT_sb[:, jt * P:(jt + 1) * P],
                    rhs=wT_sb,
                    start=(jt == 0),
                    stop=False,
                )
                nc.tensor.matmul(
                    h_all[:, jt * H:(jt + 1) * H],
                    lhsT=ones_row,
                    rhs=bias_sb,
                    start=False,
                    stop=(jt == NJT - 1),
                )

            # m_rep[:, jt, r, :] = m[jt*128:(jt+1)*128, :] for every r (IC copies)
            m_rep = consts.tile([P, NJT, IC, H], F16)
            for jt in range(NJT):
                h_ps = h_all[:, jt * H:(jt + 1) * H]
                stats = small.tile([P, 6], F32)
                nc.vector.bn_stats(stats, h_ps)
                mv = small.tile([P, 2], F32)
                nc.vector.bn_aggr(mv, stats)
                xc = small.tile([P, H], F32)
                nc.vector.tensor_scalar_sub(xc, h_ps, mv[:, 0:1])
                stdv = small.tile([P, 1], F32)
                nc.scalar.activation(
                    stdv, mv[:, 1:2], mybir.ActivationFunctionType.Sqrt, bias=eps_t
                )
                rstd = small.tile([P, 1], F32)
                nc.vector.reciprocal(rstd, stdv)
                xn = small.tile([P, H], F32)
                nc.vector.tensor_scalar_mul(xn, xc, rstd)
                sg = small.tile([P, H], F32)
                nc.scalar.activation(sg, xn, mybir.ActivationFunctionType.Sigmoid)
                nc.vector.tensor_mul(m_rep[:, jt, 0, :], xn, sg)
                rep = 1
                while rep < IC:
                    cnt = min(rep, IC - rep)
                    nc.vector.tensor_copy(
                        m_rep[:, jt, rep:rep + cnt, :], m_rep[:, jt, 0:cnt, :]
                    )
                    rep += cnt

            # stage-1 PSUM pool stays open: releasing it would put release
            # waits on stage-2 Matmults, which walrus cannot encode.
            # ------------- out[i,h] = sum_j mask[j,i,h] * m[j,h] -------------
            # acc_c[k, f] += sel[:, 0:k+1].T-routed column sum of the product
            # tile over the j partition axis.  One accumulation group per PSUM
            # bank spans the whole loop -> no mid-loop release waits.
            opsum_cm = tc.tile_pool(name="opsum", bufs=1, space="PSUM")
            opsum = opsum_cm.__enter__()
            accs = [
                opsum.tile([P, MMF], F32, name=f"acc{c}", tag=f"acc{c}")
                for c in range(CH // MMF)
            ]
            for k in range(nch):
                mt = pre_mts[k] if k < nbuf else issue_mask_load(k)
                # all-16-bit streams: DVE tensor_mul runs in 2x packed mode
                # and the PE streams fp16 moving operands at full rate; DVE
                # computes fp32 internally and PSUM accumulation stays fp32.
                pt = prods.tile([P, NJT * CH], F16)
                for jt in range(NJT):
                    # per-jt multiply: finer DVE->PE pipelining, and PE gets
                    # work every ~2us which keeps the HAM clock-gate warm
                    nc.vector.tensor_mul(
                        pt[:, jt * CH:(jt + 1) * CH],
                        mt[:, jt * CH:(jt + 1) * CH],
                        m_rep[:, jt, :, :].rearrange("p a b -> p (a b)"),
                    )
                    for c in range(CH // MMF):
                        nc.tensor.matmul(
                            accs[c][0:nch, :],
                            lhsT=sel[:, k * nch:(k + 1) * nch],
                            rhs=pt[:, jt * CH + c * MMF:jt * CH + (c + 1) * MMF],
                            start=(k == 0 and jt == 0),
                            stop=(k == nch - 1 and jt == NJT - 1),
                        )
            # epilogue: PSUM partition k, column (i_loc, h) -> out row k*IC+i_loc
            o_sb = outs.tile([nch, CH], F32)
            for c in range(CH // MMF):
                nc.scalar.copy(o_sb[:, c * MMF:(c + 1) * MMF], accs[c][0:nch, :])
            nc.sync.dma_start(
                out=out_d[:, :].rearrange("(k i) h -> k (i h)", i=IC), in_=o_sb
            )
            opsum_cm.__exit__(None, None, None)
            stage1_psum.__exit__(None, None, None)
    nc.finalize()
    return nc


_NC_CACHE = {}


def _get_nc():
    key = "main"
    if key not in _NC_CACHE:
        _NC_CACHE[key] = build_nc()
    return _NC_CACHE[key]


def kernel(s, ef_mask, W, b):
    s = np.ascontiguousarray(s, dtype=np.float32)
    W = np.ascontiguousarray(W, dtype=np.float32)
    b = np.ascontiguousarray(b, dtype=np.float32)

    nc = _get_nc()
    in_maps = []
    for c in range(N_CORES):
        bb = c // 2
        half = c % 2
        # fp16 [i, j, h] -> [k, p, jt*ic*h]: chunk-k DMAs are fully
        # sequential in HBM with 16 KiB contiguous per-partition lines
        shard = (
            np.asarray(ef_mask[bb, half * IH:(half + 1) * IH], dtype=np.float16)
            .transpose(1, 0, 2)                    # [j, i, h]
            .reshape(NJT, P, IH // IC, IC, H)      # [jt, p, k, ic, h]
            .transpose(2, 1, 0, 3, 4)              # [k, p, jt, ic, h]
            .reshape(IH // IC, P, NJT * IC * H)
        )
        in_maps.append(
            {"s": s[bb], "w": W, "b": b, "mask": np.ascontiguousarray(shard)}
        )
    res = run_bass_kernel_spmd(nc, in_maps, list(range(N_CORES))).results
    out = np.empty((B, N, H), dtype=np.float32)
    for c in range(N_CORES):
        bb = c // 2
        half = c % 2
        out[bb, half * IH:(half + 1) * IH] = res[c]["out"]
    return out
